# revision 8
# baseline (speedup 1.0000x reference)
"""Trainium2 Bass kernel for nn_Loss_20933670601009 (gathered-prob NLL loss).

Strategy: the loss only touches 3 elements per (l, b) position (one gathered
prob from each of rule/token/reference tables), so instead of streaming the
full ~566MB of prob tensors through the cores, each core fetches just the
lines it needs from HBM and reduces them on-chip.

v3 design (vs 36.5us baseline = 12 serialized single-element indirect DMAs,
~1.1us SWDGE descriptor-gen each):
  - TWO dma_gather instructions (InstDMAGatherAnt, the production multi-index
    SWDGE gather) instead of 12 indirect DMAs:
      * token: bf16 copy of the table, rows of 512 bf16 (1KB); row index
        (q*32000 + idx)//512 <= 31999 fits the gather's int16 index type.
      * rule+ref: one f32 region of 64-elem (256B) rows; rule rows 0:16384,
        ref rows 16384:20480.
  - The gathered line holds the target at a host-known residual; a host-built
    one-hot multiply + free-axis reduce selects it (the one-hot has exactly
    one nonzero per line, so the bf16 reduce is exact up to the bf16 value).
  - Validity (gt == -1) folds into the one-hots (all-zero row -> 0, matching
    the reference's eye(V+1) trick). The mask stays an explicit multiply.
  - eps fused into the Ln activation bias: reference computes
    log(p + (p<eps)*eps); we compute log(p+eps). Identical when p < eps;
    for p >= eps the deviation is <= eps/p per position -- negligible for
    uniform-random probs.
  - ACT table load hoisted off the critical path via an early dummy Ln.
  - Partition reduction via one [128,1]x[128,1] PE matmul with weight -1/B.

Numerics: token probs ride in bf16 (rel err <= 2^-8 per element, random
sign); the loss averages ~2k of them so the final rel err is ~1e-4.

Sharding: data-parallel over L_a (128 rows -> 16 rows x 8 cores, 512
positions per core; position k maps to SBUF slot [k%128, k//128]). Per-core
partial sums are combined on the host.
"""

import os
import sys

import numpy as np

for _p in ("/opt/trn_rl_repo", "/root/.axon_site/_ro/trn_rl_repo"):
    if os.path.isdir(_p) and _p not in sys.path:
        sys.path.insert(0, _p)

L_A, B = 128, 32
V_RULE, V_TOK, V_REF = 2048, 32000, 512
EPS = 1e-07
N_CORES = 8
L_SH = L_A // N_CORES            # 16 sequence rows per core
NPOS = L_SH * B                  # 512 positions per core
P = 128                          # SBUF partitions
J = NPOS // P                    # 4 position chunks per partition

TOK_ROW = 512                    # bf16 elems per token-table row (1KB)
N_TOK_ROWS = NPOS * V_TOK // TOK_ROW          # 32000 rows
RR_ROW = 64                      # f32 elems per rule/ref row (256B)
N_RULE_ROWS = NPOS * V_RULE // RR_ROW         # 16384
N_REF_ROWS = NPOS * V_REF // RR_ROW           # 4096
N_RR_ROWS = N_RULE_ROWS + N_REF_ROWS          # 20480

_CACHE = {}


def _wrap16(arr):
    """Lay out a gather index stream in the SWDGE idx format: idx k at
    [partition k%16, slot k//16], replicated across the 8 partition groups."""
    w = arr.reshape(-1, 16).T          # [16, n/16]
    return np.tile(w, (8, 1)).astype(np.int16)


def _build():
    """Build + compile the per-core Bass module (same NEFF on all 8 cores)."""
    import concourse.bacc as bacc
    import concourse.mybir as mybir
    import concourse.tile as tile

    f32 = mybir.dt.float32
    bf16 = mybir.dt.bfloat16
    i32 = mybir.dt.int32
    i16 = mybir.dt.int16

    nc = bacc.Bacc(
        "TRN2",
        target_bir_lowering=False,
        debug=False,
        enable_asserts=False,
        num_devices=N_CORES,
        # The two real gathers emit ~64/128KB of descriptors; the default 16KB
        # SWDGE ring forces repeated emit->drain->reclaim cycles that throttle
        # descriptor generation ~4-8x.
        dynamic_dma_scratch_size=131072,
    )

    idx_d = nc.dram_tensor("idx16", [P, 96], i16, kind="ExternalInput").ap()
    mask_d = nc.dram_tensor("maskf", [P, J], i32, kind="ExternalInput").ap()
    ohT_d = nc.dram_tensor("oh_tok", [P, J, TOK_ROW], bf16, kind="ExternalInput").ap()
    ohR_d = nc.dram_tensor("oh_rr", [P, 2 * J, RR_ROW], f32, kind="ExternalInput").ap()
    tok_d = nc.dram_tensor("tok_t", [N_TOK_ROWS, TOK_ROW], bf16, kind="ExternalInput").ap()
    rr_d = nc.dram_tensor("rr_t", [N_RR_ROWS, RR_ROW], f32, kind="ExternalInput").ap()
    out_d = nc.dram_tensor("out", [1, 1], f32, kind="ExternalOutput").ap()

    with tile.TileContext(nc) as tc:
        with (
            tc.tile_pool(name="sb", bufs=1) as pool,
            tc.tile_pool(name="ps", bufs=1, space="PSUM") as psum,
        ):
            # Constants + ACT-table hoist, all dependency-free -> run early.
            negw = pool.tile([P, 1], f32)
            nc.gpsimd.memset(negw[:], -1.0 / B)
            epsb = pool.tile([P, 1], f32)
            nc.gpsimd.memset(epsb[:], EPS)
            dummy = pool.tile([P, 1], f32)
            nc.scalar.activation(
                out=dummy[:], in_=epsb[:], func=mybir.ActivationFunctionType.Ln
            )

            # Dependency-free dummy gather: absorbs the one-time custom-ucode
            # library/setup cost (~7.6us measured) while the idx DMA is in
            # flight, so the real gathers start promptly.
            zidx = pool.tile([P, 1], i16)
            nc.gpsimd.memset(zidx[:], 0)
            gZ = pool.tile([P, 1, RR_ROW], f32)
            nc.gpsimd.dma_gather(gZ[:], rr_d[:], zidx[:], 16, 16, RR_ROW)

            idx = pool.tile([P, 96], i16)
            nc.sync.dma_start(out=idx[:], in_=idx_d[:])
            mk = pool.tile([P, J], i32)
            nc.sync.dma_start(out=mk[:], in_=mask_d[:])
            ohT = pool.tile([P, J, TOK_ROW], bf16)
            nc.scalar.dma_start(out=ohT[:], in_=ohT_d[:])
            ohR = pool.tile([P, 2 * J, RR_ROW], f32)
            nc.scalar.dma_start(out=ohR[:], in_=ohR_d[:])

            # Token gather first: its transfer (512KB) is the longest.
            gT = pool.tile([P, J, TOK_ROW], bf16)
            nc.gpsimd.dma_gather(gT[:], tok_d[:], idx[:, 0:32], NPOS, NPOS, TOK_ROW)
            gR = pool.tile([P, 2 * J, RR_ROW], f32)
            nc.gpsimd.dma_gather(gR[:], rr_d[:], idx[:, 32:96], 2 * NPOS, 2 * NPOS, RR_ROW)

            # One-hot select: exactly one nonzero per line.
            mT = pool.tile([P, J, TOK_ROW], bf16)
            nc.vector.tensor_mul(out=mT[:], in0=gT[:], in1=ohT[:])
            selT = pool.tile([P, J], f32)
            nc.vector.reduce_sum(out=selT[:], in_=mT[:], axis=mybir.AxisListType.X)
            mR = pool.tile([P, 2 * J, RR_ROW], f32)
            nc.vector.tensor_mul(out=mR[:], in0=gR[:], in1=ohR[:])
            selR = pool.tile([P, 2 * J], f32)
            nc.vector.reduce_sum(out=selR[:], in_=mR[:], axis=mybir.AxisListType.X)

            s = pool.tile([P, J], f32)
            nc.vector.tensor_add(out=s[:], in0=selT[:], in1=selR[:, 0:J])
            nc.vector.tensor_add(out=s[:], in0=s[:], in1=selR[:, J:2 * J])

            ln = pool.tile([P, J], f32)
            nc.scalar.activation(
                out=ln[:], in_=s[:], func=mybir.ActivationFunctionType.Ln,
                bias=epsb[:],
            )
            lm = pool.tile([P, J], f32)
            nc.vector.tensor_mul(out=lm[:], in0=ln[:], in1=mk[:].bitcast(f32))
            rs = pool.tile([P, 1], f32)
            nc.vector.reduce_sum(out=rs[:], in_=lm[:], axis=mybir.AxisListType.X)

            acc = psum.tile([1, 1], f32)
            nc.tensor.matmul(out=acc[:], lhsT=rs[:], rhs=negw[:], start=True, stop=True)
            res = pool.tile([1, 1], f32)
            nc.scalar.copy(out=res[:], in_=acc[:])
            nc.sync.dma_start(out=out_d[:], in_=res[:])

    nc.compile()
    return nc


def get_nc():
    if "nc" not in _CACHE:
        _CACHE["nc"] = _build()
    return _CACHE["nc"]


def make_in_maps(rule_probs, token_probs, reference_probs, ground_truth_actions, mask):
    """Shard the full inputs into 8 per-core input maps."""
    import ml_dtypes

    bf16 = ml_dtypes.bfloat16
    rule_probs = np.ascontiguousarray(np.asarray(rule_probs, dtype=np.float32))
    token_probs = np.ascontiguousarray(np.asarray(token_probs, dtype=np.float32))
    reference_probs = np.ascontiguousarray(np.asarray(reference_probs, dtype=np.float32))
    gt = np.asarray(ground_truth_actions, dtype=np.int32)
    mask = np.asarray(mask, dtype=np.int32)

    q = np.arange(NPOS, dtype=np.int64)
    kk = np.arange(NPOS, dtype=np.int64)
    in_maps = []
    for i in range(N_CORES):
        lo, hi = i * L_SH, (i + 1) * L_SH
        gt_sh = gt[lo:hi].reshape(NPOS, 3).astype(np.int64)
        m_sh = mask[lo:hi].reshape(NPOS)

        off_t = q * V_TOK + np.clip(gt_sh[:, 1], 0, V_TOK - 1)
        off_r = q * V_RULE + np.clip(gt_sh[:, 0], 0, V_RULE - 1)
        off_f = q * V_REF + np.clip(gt_sh[:, 2], 0, V_REF - 1)
        idx_t, r_t = np.divmod(off_t, TOK_ROW)
        idx_r, r_r = np.divmod(off_r, RR_ROW)
        idx_f, r_f = np.divmod(off_f, RR_ROW)
        idx_f += N_RULE_ROWS

        idx16 = np.empty((P, 96), np.int16)
        idx16[:, 0:32] = _wrap16(idx_t)
        idx16[:, 32:96] = _wrap16(np.concatenate([idx_r, idx_f]))

        oh_tok = np.zeros((P, J, TOK_ROW), bf16)
        v = gt_sh[:, 1] >= 0
        oh_tok[kk[v] % P, kk[v] // P, r_t[v]] = bf16(1.0)
        oh_rr = np.zeros((P, 2 * J, RR_ROW), np.float32)
        v = gt_sh[:, 0] >= 0
        oh_rr[kk[v] % P, kk[v] // P, r_r[v]] = 1.0
        v = gt_sh[:, 2] >= 0
        k2 = kk[v] + NPOS
        oh_rr[k2 % P, k2 // P, r_f[v]] = 1.0

        maskf = (
            m_sh.reshape(J, P).T.astype(np.float32).copy().view(np.int32)
        )

        tok_t = token_probs[lo:hi].reshape(-1).astype(bf16).reshape(N_TOK_ROWS, TOK_ROW)
        rr_t = np.concatenate(
            [rule_probs[lo:hi].reshape(-1), reference_probs[lo:hi].reshape(-1)]
        ).reshape(N_RR_ROWS, RR_ROW)

        in_maps.append(
            {
                "idx16": idx16,
                "maskf": maskf,
                "oh_tok": oh_tok,
                "oh_rr": oh_rr,
                "tok_t": tok_t,
                "rr_t": rr_t,
            }
        )
    return in_maps


def run(inputs, trace=False, trace_cores=None):
    """Run on the 8 NeuronCores; returns (scalar ndarray, BassKernelResults)."""
    from concourse.bass_utils import run_bass_kernel_spmd

    nc = get_nc()
    in_maps = make_in_maps(**inputs)
    res = run_bass_kernel_spmd(
        nc,
        in_maps,
        core_ids=list(range(N_CORES)),
        trace=trace,
        trace_cores=trace_cores,
    )
    total = np.float64(0.0)
    for r in res.results:
        total += np.float64(r["out"].reshape(())[()])
    return np.asarray(total, dtype=np.float32), res


def kernel(**inputs) -> np.ndarray:
    out, _ = run(inputs)
    return out


# revision 9
# speedup vs baseline: 1.4379x; 1.4379x over previous
"""Trainium2 Bass kernel for nn_Loss_20933670601009 (gathered-prob NLL loss).

Strategy: the loss only touches 3 elements per (l, b) position (one gathered
prob from each of rule/token/reference tables), and only for positions with
mask == 1 (~52%). Each core element-gathers exactly the values it needs from
HBM with single-element indirect DMAs, then runs a short fused reduce.

v5 vs the 36.5us baseline (12 serialized [128,1] indirect gathers + unfused
tail):
  - Mask compaction: masked-out positions contribute exactly 0, so they are
    never gathered (or even uploaded). Unmasked positions are dealt evenly
    across the 8 cores (266/core here), so each component needs
    K = ceil(266/128) = 3 gather columns -> 9 indirect DMAs instead of 12.
  - Offsets fully precomputed on host (int32 flat indices); invalid
    components (gt == -1) point at a 0.0 sentinel (exact), padding slots
    point all three components at a (1-eps)/3 sentinel so ln(sum+eps) ==
    ln(1.0) == 0 (error ~4e-8/slot).
  - eps fused into the Ln bias: reference computes log(p + (p<eps)*eps); we
    compute log(p+eps) -- identical when p < eps, deviation <= eps/p
    otherwise (negligible for uniform probs).
  - Row-sum fused into the Ln activation's accum_out; no mask multiply on
    device at all; partition reduction via one [128,1]x[128,1] PE matmul
    with weight -1/B.
  - ACT table load hoisted off the critical path via an early dummy Ln.

Per-core partial sums are combined on the host.
"""

import os
import sys

import numpy as np

for _p in ("/opt/trn_rl_repo", "/root/.axon_site/_ro/trn_rl_repo"):
    if os.path.isdir(_p) and _p not in sys.path:
        sys.path.insert(0, _p)

L_A, B = 128, 32
V_RULE, V_TOK, V_REF = 2048, 32000, 512
VSUM = V_RULE + V_TOK + V_REF
EPS = 1e-07
N_CORES = 8
P = 128

_CACHE = {}


def _build(K):
    """Build + compile the per-core Bass module for K gather columns per
    component (n_pad = 128*K compacted slots per core)."""
    import concourse.bacc as bacc
    import concourse.bass as bass
    import concourse.mybir as mybir
    import concourse.tile as tile

    f32 = mybir.dt.float32
    i32 = mybir.dt.int32

    n_pad = P * K
    n_flat = n_pad * VSUM

    nc = bacc.Bacc(
        "TRN2",
        target_bir_lowering=False,
        debug=False,
        enable_asserts=False,
        num_devices=N_CORES,
    )

    # meta cols: c*K+k holds the full-flat offset of component c (0=rule,
    # 1=ref, 2=token) for slot k*128+p.
    meta_d = nc.dram_tensor("meta", [P, 3 * K], i32, kind="ExternalInput").ap()
    flat_d = nc.dram_tensor("probs_flat", [n_flat + 16, 1], f32, kind="ExternalInput").ap()
    out_d = nc.dram_tensor("out", [1, 1], f32, kind="ExternalOutput").ap()

    with tile.TileContext(nc) as tc:
        with (
            tc.tile_pool(name="sb", bufs=1) as pool,
            tc.tile_pool(name="ps", bufs=1, space="PSUM") as psum,
        ):
            negw = pool.tile([P, 1], f32)
            nc.gpsimd.memset(negw[:], -1.0 / B)
            epsb = pool.tile([P, 1], f32)
            nc.gpsimd.memset(epsb[:], EPS)
            # Hoists the Ln ACT table load (1.3us) off the critical path.
            dummy = pool.tile([P, 1], f32)
            nc.scalar.activation(
                out=dummy[:], in_=epsb[:], func=mybir.ActivationFunctionType.Ln
            )

            meta = pool.tile([P, 3 * K], i32)
            nc.sync.dma_start(out=meta[:], in_=meta_d[:])

            g = pool.tile([P, 3 * K], f32)
            for col in range(3 * K):
                nc.gpsimd.indirect_dma_start(
                    out=g[:, col:col + 1],
                    out_offset=None,
                    in_=flat_d[:],
                    in_offset=bass.IndirectOffsetOnAxis(
                        ap=meta[:, col:col + 1], axis=0
                    ),
                    element_offset=0,
                )
                if col == 2 * K - 1:
                    # rule + ref partial overlaps the token gathers
                    part = pool.tile([P, K], f32)
                    nc.vector.tensor_add(
                        out=part[:], in0=g[:, 0:K], in1=g[:, K:2 * K]
                    )

            s = pool.tile([P, K], f32)
            nc.vector.tensor_add(out=s[:], in0=part[:], in1=g[:, 2 * K:3 * K])

            # rs[p] = sum_k ln(s[p,k] + eps)
            ln = pool.tile([P, K], f32)
            rs = pool.tile([P, 1], f32)
            nc.scalar.activation(
                out=ln[:], in_=s[:], func=mybir.ActivationFunctionType.Ln,
                bias=epsb[:], accum_out=rs[:],
            )

            acc = psum.tile([1, 1], f32)
            nc.tensor.matmul(out=acc[:], lhsT=rs[:], rhs=negw[:], start=True, stop=True)
            res = pool.tile([1, 1], f32)
            nc.scalar.copy(out=res[:], in_=acc[:])
            nc.sync.dma_start(out=out_d[:], in_=res[:])

    nc.compile()
    return nc


def get_nc(K):
    key = ("nc", K)
    if key not in _CACHE:
        _CACHE[key] = _build(K)
    return _CACHE[key]


def make_in_maps(rule_probs, token_probs, reference_probs, ground_truth_actions, mask):
    """Deal unmasked positions evenly across 8 cores; build per-core inputs."""
    rule_probs = np.asarray(rule_probs, dtype=np.float32).reshape(-1, V_RULE)
    token_probs = np.asarray(token_probs, dtype=np.float32).reshape(-1, V_TOK)
    reference_probs = np.asarray(reference_probs, dtype=np.float32).reshape(-1, V_REF)
    gt = np.asarray(ground_truth_actions, dtype=np.int32).reshape(-1, 3)
    m = np.asarray(mask, dtype=np.int32).reshape(-1).astype(bool)

    pos = np.nonzero(m)[0]                      # unmasked global positions
    n_max = -(-len(pos) // N_CORES) if len(pos) else 1
    K = max(1, -(-n_max // P))
    n_pad = P * K
    n_flat = n_pad * VSUM
    ZERO_IDX = n_flat                           # sentinel 0.0
    ONE3_IDX = n_flat + 1                       # sentinel (1-eps)/3

    in_maps = []
    for i in range(N_CORES):
        mine = pos[i::N_CORES]                  # dealt round-robin
        n = len(mine)
        gt_c = gt[mine].astype(np.int64)        # [n, 3]

        # per-slot flat offsets; layout: [rule rows | token rows | ref rows]
        j = np.arange(n, dtype=np.int64)
        off_rule = j * V_RULE + np.clip(gt_c[:, 0], 0, V_RULE - 1)
        off_tok = n_pad * V_RULE + j * V_TOK + np.clip(gt_c[:, 1], 0, V_TOK - 1)
        off_ref = (
            n_pad * (V_RULE + V_TOK) + j * V_REF + np.clip(gt_c[:, 2], 0, V_REF - 1)
        )
        off_rule = np.where(gt_c[:, 0] >= 0, off_rule, ZERO_IDX)
        off_tok = np.where(gt_c[:, 1] >= 0, off_tok, ZERO_IDX)
        off_ref = np.where(gt_c[:, 2] >= 0, off_ref, ZERO_IDX)

        meta = np.full((P, 3 * K), ONE3_IDX, np.int32)
        for c, offs in enumerate((off_rule, off_ref, off_tok)):
            cols = np.full(n_pad, ONE3_IDX, np.int64)
            cols[:n] = offs
            meta[:, c * K:(c + 1) * K] = cols.reshape(K, P).T

        flat = np.empty(n_flat + 16, np.float32)
        flat[:n * V_RULE] = rule_probs[mine].reshape(-1)
        flat[n_pad * V_RULE:n_pad * V_RULE + n * V_TOK] = token_probs[mine].reshape(-1)
        flat[n_pad * (V_RULE + V_TOK):n_pad * (V_RULE + V_TOK) + n * V_REF] = (
            reference_probs[mine].reshape(-1)
        )
        flat[ZERO_IDX] = 0.0
        flat[ONE3_IDX] = (1.0 - EPS) / 3.0

        in_maps.append({"meta": meta, "probs_flat": flat.reshape(-1, 1)})
    return K, in_maps


def run(inputs, trace=False, trace_cores=None):
    """Run on the 8 NeuronCores; returns (scalar ndarray, BassKernelResults)."""
    from concourse.bass_utils import run_bass_kernel_spmd

    K, in_maps = make_in_maps(**inputs)
    nc = get_nc(K)
    res = run_bass_kernel_spmd(
        nc,
        in_maps,
        core_ids=list(range(N_CORES)),
        trace=trace,
        trace_cores=trace_cores,
    )
    total = np.float64(0.0)
    for r in res.results:
        total += np.float64(r["out"].reshape(())[()])
    return np.asarray(total, dtype=np.float32), res


def kernel(**inputs) -> np.ndarray:
    out, _ = run(inputs)
    return out
